# revision 16
# baseline (speedup 1.0000x reference)
"""MoE layer (B=2,S=2048,D=1024,H=2048,E=8,top-2) on 8 Trainium2 NeuronCores.

Strategy: expert-parallel. The gate (N x E logits, top-2, softmax) is tiny and
runs on host. Core e owns expert e's weights; the host gathers the tokens
routed to expert e, ships them (transposed, padded to capacity C) to core e,
each core runs the dense FFN y = silu(x @ w1.T + b1) @ w2.T + b2 on its
tokens, and the host scatter-adds gate-weighted results back.

Device layout (per core):
  xT   [D, C]   routed tokens, transposed  (lhs-free layout for matmul 1)
  w1T  [D, H]   w1[e].T
  w2T  [H, D]   w2[e].T
  b1   [128, H/128]  b1[e] chunked per-partition
  b2   [128, D/128]
  yT   [D, C]   output, transposed

Matmul 1: hT[m*128+p, n] = sum_d w1[h,d] x[n,d]  -> stationary w1T chunk,
moving xT chunk, PSUM accumulate over D. Silu+bias fused on ScalarE.
Matmul 2 symmetric with w2T/hT -> yT, Identity+bias epilogue.
All matmuls run as float32r (full PE rate for moving dim >= 256).
"""

import math
from contextlib import ExitStack

import numpy as np

import concourse.bass as bass
import concourse.tile as tile
from concourse import bacc, mybir
from concourse.bass_utils import run_bass_kernel_spmd

B, S, D, H, E, TOPK = 2, 2048, 1024, 2048, 8, 2
N = B * S
P = 128
TT = 512  # token tile (moving dim per matmul)
KD, KH = D // P, H // P
N_CORES = 8
BALANCE_COEFF = 0.01

_kernel_cache: dict[int, object] = {}
_TRACE = False
LAST_RESULTS = None


def _build(C: int):
    f32 = mybir.dt.float32
    f32r = mybir.dt.float32r
    act = mybir.ActivationFunctionType

    nc = bacc.Bacc("TRN2", target_bir_lowering=False, debug=False)
    xT = nc.dram_tensor("xT", [D, C], f32r, kind="ExternalInput")
    w1T = nc.dram_tensor("w1T", [D, H], f32r, kind="ExternalInput")
    w2T = nc.dram_tensor("w2T", [H, D], f32r, kind="ExternalInput")
    b1 = nc.dram_tensor("b1", [P, KH], f32, kind="ExternalInput")
    b2 = nc.dram_tensor("b2", [P, KD], f32, kind="ExternalInput")
    yT = nc.dram_tensor("yT", [D, C], f32, kind="ExternalOutput")

    xT_r = xT.rearrange("(k p) c -> k p c", p=P)
    w1T_r = w1T.rearrange("(k p) h -> k p h", p=P)
    w2T_r = w2T.rearrange("(k p) d -> k p d", p=P)
    yT_r = yT.rearrange("(k p) c -> k p c", p=P)

    # token tiles: mostly 512-wide, optional trailing 256 (C is a multiple
    # of 256; fp32r matmul needs moving dim >= 256 for full PE rate)
    assert C % 256 == 0
    tiles = [TT] * (C // TT)
    if C % TT:
        tiles.append(C % TT)
    G = 4  # PSUM bank group: h/d chunks accumulated concurrently

    with tile.TileContext(nc) as tc, ExitStack() as ctx:
        wpool = ctx.enter_context(tc.tile_pool(name="w", bufs=1))
        bpool = ctx.enter_context(tc.tile_pool(name="b", bufs=1))
        xpool = ctx.enter_context(tc.tile_pool(name="x", bufs=1))
        hpool = ctx.enter_context(tc.tile_pool(name="h", bufs=1))
        ypool = ctx.enter_context(tc.tile_pool(name="y", bufs=4))
        pspool = ctx.enter_context(tc.tile_pool(name="ps", bufs=8, space="PSUM"))

        # x tile 0 + w1 first, all as separate chunk DMAs on the sync HWDGE
        # (independent DMAs fan out across parallel hardware queues); w2
        # follows on the same path. Biases ride the gpsimd queue.
        # Issue order matters: the sync sequencer issues one DMA per ~0.7us,
        # and matmul k of group 0 is gated on xt chunk k + the first half of
        # w1 chunk k. Interleave so the PE unblocks progressively, then
        # stream the rest (w1 second halves, w2) behind it.
        xt = xpool.tile([P, KD * TT], f32r)
        tt0 = tiles[0]
        HW2, DW2 = H // 2, D // 2
        w1s = [wpool.tile([P, H], f32r, tag=f"w1_{k}", name=f"w1_{k}") for k in range(KD)]
        w2s = [wpool.tile([P, D], f32r, tag=f"w2_{k}", name=f"w2_{k}") for k in range(KH)]
        for k in range(KD):
            nc.sync.dma_start(
                xt[:, k * TT : k * TT + tt0], xT_r[k, :, 0:tt0]
            )
            nc.sync.dma_start(w1s[k][:, :HW2], w1T_r[k, :, :HW2])
        b1t = bpool.tile([P, KH], f32, tag="b1")
        nc.gpsimd.dma_start(b1t[:], b1[:])
        b2t = bpool.tile([P, KD], f32, tag="b2")
        nc.gpsimd.dma_start(b2t[:], b2[:])
        for k in range(KD):
            nc.sync.dma_start(w1s[k][:, HW2:], w1T_r[k, :, HW2:])
        for k in range(KH):
            nc.sync.dma_start(w2s[k][:, :DW2], w2T_r[k, :, :DW2])
            nc.sync.dma_start(w2s[k][:, DW2:], w2T_r[k, :, DW2:])

        # Warm-up matmuls: the PE idles ~13us waiting for the first x/w1
        # chunks; junk matmuls on a memset tile get the HAM clock gate to
        # 8/8 (2.4 GHz) before real work arrives, avoiding the ~2x-slow
        # cold phase. Results go to a never-read PSUM slot.
        warm_src = bpool.tile([P, TT], mybir.dt.bfloat16, tag="warm")
        nc.vector.memset(warm_src[:], 0.0)
        warm_ps = pspool.tile([P, TT], f32, tag="ps", name="warm_ps")
        for _ in range(20):
            nc.tensor.matmul(
                warm_ps[:], warm_src[:, :P], warm_src[:],
                start=True, stop=True,
            )

        off = 0
        for ti, tt in enumerate(tiles):
            if ti > 0:
                xt = xpool.tile([P, KD * TT], f32r)
                for k in range(KD):
                    nc.sync.dma_start(
                        xt[:, k * TT : k * TT + tt],
                        xT_r[k, :, off : off + tt],
                    )
            ht = hpool.tile([P, KH * TT], f32r)
            # mm1: k-outer so the first matmuls only need w1s[0] + xt chunk 0
            for g in range(KH // G):
                pss = [pspool.tile([P, TT], f32, tag="ps", name=f"ps_{ti}_{g}_{i}") for i in range(G)]
                for k in range(KD):
                    for mi in range(G):
                        m = g * G + mi
                        nc.tensor.matmul(
                            pss[mi][:, :tt],
                            w1s[k][:, m * P : (m + 1) * P],
                            xt[:, k * TT : k * TT + tt],
                            start=(k == 0),
                            stop=(k == KD - 1),
                        )
                for mi in range(G):
                    m = g * G + mi
                    nc.scalar.activation(
                        ht[:, m * TT : m * TT + tt], pss[mi][:, :tt],
                        act.Silu, bias=b1t[:, m : m + 1],
                    )
            # mm2
            for g in range(KD // G):
                pss = [pspool.tile([P, TT], f32, tag="ps", name=f"ps_{ti}_{g}_{i}") for i in range(G)]
                for k in range(KH):
                    for ji in range(G):
                        j = g * G + ji
                        nc.tensor.matmul(
                            pss[ji][:, :tt],
                            w2s[k][:, j * P : (j + 1) * P],
                            ht[:, k * TT : k * TT + tt],
                            start=(k == 0),
                            stop=(k == KH - 1),
                        )
                for ji in range(G):
                    j = g * G + ji
                    yt = ypool.tile([P, TT], f32)
                    nc.scalar.activation(
                        yt[:, :tt], pss[ji][:, :tt], act.Identity,
                        bias=b2t[:, j : j + 1],
                    )
                    nc.gpsimd.dma_start(
                        yT_r[j, :, off : off + tt], yt[:, :tt]
                    )
            off += tt

    nc.compile()
    return nc


def _get_kernel(C: int):
    if C not in _kernel_cache:
        _kernel_cache[C] = _build(C)
    return _kernel_cache[C]


def kernel(**inputs):
    global LAST_RESULTS
    x = np.asarray(inputs["x"], dtype=np.float32)
    gate_w = np.asarray(inputs["gate_w"], dtype=np.float32)
    w1 = np.asarray(inputs["w1"], dtype=np.float32)
    b1 = np.asarray(inputs["b1"], dtype=np.float32)
    w2 = np.asarray(inputs["w2"], dtype=np.float32)
    b2 = np.asarray(inputs["b2"], dtype=np.float32)

    xf = x.reshape(N, D)

    # --- gating on host (tiny) ---
    logits = xf.astype(np.float64) @ gate_w.T.astype(np.float64)  # [N, E]
    top_i = np.argsort(-logits, axis=1, kind="stable")[:, :TOPK]  # [N, K]
    top_v = np.take_along_axis(logits, top_i, axis=1)
    ex = np.exp(top_v - top_v[:, :1])
    gates = (ex / ex.sum(axis=1, keepdims=True)).astype(np.float32)  # [N, K]

    counts = np.bincount(top_i.ravel(), minlength=E).astype(np.float64)
    load = counts / (N * TOPK)
    aux_loss = np.float32(BALANCE_COEFF * E * np.sum(load * load))

    # --- dispatch ---
    idx_e, g_e = [], []
    for e in range(E):
        mask = top_i == e  # at most one True per row
        rows = np.nonzero(mask.any(axis=1))[0]
        idx_e.append(rows)
        g_e.append(gates[mask])  # row-major matches ascending rows

    # Capacity: smallest multiple of 256 (>=512) such that at most
    # OVERFLOW_MAX (token, expert) pairs spill; spilled pairs are computed
    # on host in fp32 (exact). Balanced loads make spill tiny or zero.
    OVERFLOW_MAX = 256
    cnts = np.array([len(r) for r in idx_e])
    C = max(512, int(math.ceil(cnts.max() / 256)) * 256)
    while C > 512 and np.maximum(cnts - (C - 256), 0).sum() <= OVERFLOW_MAX:
        C -= 256
    nc = _get_kernel(C)

    in_maps = []
    for e in range(E):
        cnt = min(len(idx_e[e]), C)
        xTe = np.zeros((D, C), dtype=np.float32)
        if cnt:
            xTe[:, :cnt] = xf[idx_e[e][:cnt]].T
        in_maps.append(
            {
                "xT": xTe,
                "w1T": np.ascontiguousarray(w1[e].T),
                "w2T": np.ascontiguousarray(w2[e].T),
                "b1": np.ascontiguousarray(b1[e].reshape(KH, P).T),
                "b2": np.ascontiguousarray(b2[e].reshape(KD, P).T),
            }
        )

    res = run_bass_kernel_spmd(
        nc, in_maps, core_ids=list(range(N_CORES)), trace=_TRACE
    )
    LAST_RESULTS = res

    # --- combine on host ---
    out = np.zeros((N, D), dtype=np.float32)
    for e in range(E):
        cnt = min(len(idx_e[e]), C)
        if cnt:
            ye = res.results[e]["yT"][:, :cnt].T  # [cnt, D]
            out[idx_e[e][:cnt]] += g_e[e][:cnt, None] * ye
        if len(idx_e[e]) > C:  # overflow pairs -> exact host FFN
            rows = idx_e[e][C:]
            z = xf[rows] @ w1[e].T + b1[e]
            h = z / (1.0 + np.exp(-z))
            yo = h @ w2[e].T + b2[e]
            out[rows] += g_e[e][C:, None] * yo

    return out.reshape(B, S, D), aux_loss


# revision 18
# speedup vs baseline: 1.0733x; 1.0733x over previous
"""MoE layer (B=2,S=2048,D=1024,H=2048,E=8,top-2) on 8 Trainium2 NeuronCores.

Strategy: expert-parallel. The gate (N x E logits, top-2, softmax) is tiny and
runs on host. Core e owns expert e's weights; the host gathers the tokens
routed to expert e, ships them (transposed, padded to capacity C) to core e,
each core runs the dense FFN y = silu(x @ w1.T + b1) @ w2.T + b2 on its
tokens, and the host scatter-adds gate-weighted results back.

Device layout (per core):
  xT   [D, C]   routed tokens, transposed  (lhs-free layout for matmul 1)
  w1T  [D, H]   w1[e].T
  w2T  [H, D]   w2[e].T
  b1   [128, H/128]  b1[e] chunked per-partition
  b2   [128, D/128]
  yT   [D, C]   output, transposed

Matmul 1: hT[m*128+p, n] = sum_d w1[h,d] x[n,d]  -> stationary w1T chunk,
moving xT chunk, PSUM accumulate over D. Silu+bias fused on ScalarE.
Matmul 2 symmetric with w2T/hT -> yT, Identity+bias epilogue.
All matmuls run as float32r (full PE rate for moving dim >= 256).
"""

import math
from contextlib import ExitStack

import numpy as np

import concourse.bass as bass
import concourse.tile as tile
from concourse import bacc, mybir
from concourse.bass_utils import run_bass_kernel_spmd

B, S, D, H, E, TOPK = 2, 2048, 1024, 2048, 8, 2
N = B * S
P = 128
TT = 512  # token tile (moving dim per matmul)
KD, KH = D // P, H // P
N_CORES = 8
BALANCE_COEFF = 0.01

_kernel_cache: dict[int, object] = {}
_TRACE = False
LAST_RESULTS = None


def _build(C: int):
    f32 = mybir.dt.float32
    f32r = mybir.dt.float32r
    act = mybir.ActivationFunctionType

    nc = bacc.Bacc("TRN2", target_bir_lowering=False, debug=False)
    xT = nc.dram_tensor("xT", [D, C], f32r, kind="ExternalInput")
    w1T = nc.dram_tensor("w1T", [D, H], f32r, kind="ExternalInput")
    w2T = nc.dram_tensor("w2T", [H, D], f32r, kind="ExternalInput")
    b1 = nc.dram_tensor("b1", [P, KH], f32, kind="ExternalInput")
    b2 = nc.dram_tensor("b2", [P, KD], f32, kind="ExternalInput")
    yT = nc.dram_tensor("yT", [D, C], f32, kind="ExternalOutput")

    xT_r = xT.rearrange("(k p) c -> k p c", p=P)
    w1T_r = w1T.rearrange("(k p) h -> k p h", p=P)
    w2T_r = w2T.rearrange("(k p) d -> k p d", p=P)
    yT_r = yT.rearrange("(k p) c -> k p c", p=P)

    # token tiles: mostly 512-wide, optional trailing 256 (C is a multiple
    # of 256; fp32r matmul needs moving dim >= 256 for full PE rate)
    assert C % 256 == 0
    tiles = [TT] * (C // TT)
    if C % TT:
        tiles.append(C % TT)
    G = 4  # PSUM bank group: h/d chunks accumulated concurrently

    with tile.TileContext(nc) as tc, ExitStack() as ctx:
        wpool = ctx.enter_context(tc.tile_pool(name="w", bufs=1))
        bpool = ctx.enter_context(tc.tile_pool(name="b", bufs=1))
        xpool = ctx.enter_context(tc.tile_pool(name="x", bufs=1))
        hpool = ctx.enter_context(tc.tile_pool(name="h", bufs=1))
        ypool = ctx.enter_context(tc.tile_pool(name="y", bufs=4))
        pspool = ctx.enter_context(tc.tile_pool(name="ps", bufs=8, space="PSUM"))

        # x tile 0 + w1 first, all as separate chunk DMAs on the sync HWDGE
        # (independent DMAs fan out across parallel hardware queues); w2
        # follows on the same path. Biases ride the gpsimd queue.
        # Issue order matters: the sync sequencer issues one DMA per ~0.7us,
        # and matmul k of group 0 is gated on xt chunk k + the first half of
        # w1 chunk k. Interleave so the PE unblocks progressively, then
        # stream the rest (w1 second halves, w2) behind it.
        xt = xpool.tile([P, KD * TT], f32r)
        tt0 = tiles[0]
        HW2, DW2 = H // 2, D // 2
        w1s = [wpool.tile([P, H], f32r, tag=f"w1_{k}", name=f"w1_{k}") for k in range(KD)]
        w2s = [wpool.tile([P, D], f32r, tag=f"w2_{k}", name=f"w2_{k}") for k in range(KH)]
        for k in range(KD):
            nc.sync.dma_start(
                xt[:, k * TT : k * TT + tt0], xT_r[k, :, 0:tt0]
            )
            nc.sync.dma_start(w1s[k][:, :HW2], w1T_r[k, :, :HW2])
        b1t = bpool.tile([P, KH], f32, tag="b1")
        nc.gpsimd.dma_start(b1t[:], b1[:])
        b2t = bpool.tile([P, KD], f32, tag="b2")
        nc.gpsimd.dma_start(b2t[:], b2[:])
        for k in range(KD):
            nc.sync.dma_start(w1s[k][:, HW2:], w1T_r[k, :, HW2:])
        for k in range(KH):
            nc.sync.dma_start(w2s[k][:, :DW2], w2T_r[k, :, :DW2])
            nc.sync.dma_start(w2s[k][:, DW2:], w2T_r[k, :, DW2:])

        # Warm-up matmuls: the PE idles ~13us waiting for the first x/w1
        # chunks; junk matmuls on a memset tile get the HAM clock gate to
        # 8/8 (2.4 GHz) before real work arrives, avoiding the ~2x-slow
        # cold phase. Results go to a never-read PSUM slot.
        warm_src = bpool.tile([P, TT], mybir.dt.bfloat16, tag="warm")
        nc.vector.memset(warm_src[:], 0.0)
        warm_ps = pspool.tile([P, TT], f32, tag="ps", name="warm_ps")
        for _ in range(20):
            nc.tensor.matmul(
                warm_ps[:], warm_src[:, :P], warm_src[:],
                start=True, stop=True,
            )

        off = 0
        for ti, tt in enumerate(tiles):
            if ti > 0:
                xt = xpool.tile([P, KD * TT], f32r)
                for k in range(KD):
                    nc.sync.dma_start(
                        xt[:, k * TT : k * TT + tt],
                        xT_r[k, :, off : off + tt],
                    )
            ht = hpool.tile([P, KH * TT], f32r)
            # mm1: k-outer so the first matmuls only need w1s[0] + xt chunk 0
            for g in range(KH // G):
                pss = [pspool.tile([P, TT], f32, tag="ps", name=f"ps_{ti}_{g}_{i}") for i in range(G)]
                for k in range(KD):
                    for mi in range(G):
                        m = g * G + mi
                        nc.tensor.matmul(
                            pss[mi][:, :tt],
                            w1s[k][:, m * P : (m + 1) * P],
                            xt[:, k * TT : k * TT + tt],
                            start=(k == 0),
                            stop=(k == KD - 1),
                        )
                for mi in range(G):
                    m = g * G + mi
                    nc.scalar.activation(
                        ht[:, m * TT : m * TT + tt], pss[mi][:, :tt],
                        act.Silu, bias=b1t[:, m : m + 1],
                    )
            # mm2
            for g in range(KD // G):
                pss = [pspool.tile([P, TT], f32, tag="ps", name=f"ps_{ti}_{g}_{i}") for i in range(G)]
                for k in range(KH):
                    for ji in range(G):
                        j = g * G + ji
                        nc.tensor.matmul(
                            pss[ji][:, :tt],
                            w2s[k][:, j * P : (j + 1) * P],
                            ht[:, k * TT : k * TT + tt],
                            start=(k == 0),
                            stop=(k == KH - 1),
                        )
                for ji in range(G):
                    j = g * G + ji
                    yt = ypool.tile([P, TT], f32)
                    nc.scalar.activation(
                        yt[:, :tt], pss[ji][:, :tt], act.Identity,
                        bias=b2t[:, j : j + 1],
                    )
                    nc.gpsimd.dma_start(
                        yT_r[j, :, off : off + tt], yt[:, :tt]
                    )
            off += tt

    nc.compile()
    return nc


def _get_kernel(C: int):
    if C not in _kernel_cache:
        _kernel_cache[C] = _build(C)
    return _kernel_cache[C]


def kernel(**inputs):
    global LAST_RESULTS
    x = np.asarray(inputs["x"], dtype=np.float32)
    gate_w = np.asarray(inputs["gate_w"], dtype=np.float32)
    w1 = np.asarray(inputs["w1"], dtype=np.float32)
    b1 = np.asarray(inputs["b1"], dtype=np.float32)
    w2 = np.asarray(inputs["w2"], dtype=np.float32)
    b2 = np.asarray(inputs["b2"], dtype=np.float32)

    xf = x.reshape(N, D)

    # --- gating on host (tiny) ---
    logits = xf.astype(np.float64) @ gate_w.T.astype(np.float64)  # [N, E]
    top_i = np.argsort(-logits, axis=1, kind="stable")[:, :TOPK]  # [N, K]
    top_v = np.take_along_axis(logits, top_i, axis=1)
    ex = np.exp(top_v - top_v[:, :1])
    gates = (ex / ex.sum(axis=1, keepdims=True)).astype(np.float32)  # [N, K]

    counts = np.bincount(top_i.ravel(), minlength=E).astype(np.float64)
    load = counts / (N * TOPK)
    aux_loss = np.float32(BALANCE_COEFF * E * np.sum(load * load))

    # --- dispatch ---
    idx_e, g_e = [], []
    for e in range(E):
        mask = top_i == e  # at most one True per row
        rows = np.nonzero(mask.any(axis=1))[0]
        idx_e.append(rows)
        g_e.append(gates[mask])  # row-major matches ascending rows

    # Capacity: smallest multiple of 256 (>=512) such that at most
    # OVERFLOW_MAX (token, expert) pairs spill; spilled pairs are computed
    # on host in fp32 (exact). Balanced loads make spill tiny or zero.
    OVERFLOW_MAX = 256
    cnts = np.array([len(r) for r in idx_e])
    C = max(512, int(math.ceil(cnts.max() / 256)) * 256)
    while C > 512 and np.maximum(cnts - (C - 256), 0).sum() <= OVERFLOW_MAX:
        C -= 256
    nc = _get_kernel(C)

    in_maps = []
    for e in range(E):
        cnt = min(len(idx_e[e]), C)
        xTe = np.zeros((D, C), dtype=np.float32)
        if cnt:
            xTe[:, :cnt] = xf[idx_e[e][:cnt]].T
        in_maps.append(
            {
                "xT": xTe,
                "w1T": np.ascontiguousarray(w1[e].T),
                "w2T": np.ascontiguousarray(w2[e].T),
                "b1": np.ascontiguousarray(b1[e].reshape(KH, P).T),
                "b2": np.ascontiguousarray(b2[e].reshape(KD, P).T),
            }
        )

    res = run_bass_kernel_spmd(
        nc, in_maps, core_ids=list(range(N_CORES)), trace=_TRACE
    )
    LAST_RESULTS = res

    # --- combine on host ---
    out = np.zeros((N, D), dtype=np.float32)
    for e in range(E):
        cnt = min(len(idx_e[e]), C)
        if cnt:
            ye = res.results[e]["yT"][:, :cnt].T  # [cnt, D]
            out[idx_e[e][:cnt]] += g_e[e][:cnt, None] * ye
        if len(idx_e[e]) > C:  # overflow pairs -> exact host FFN
            rows = idx_e[e][C:]
            z = xf[rows] @ w1[e].T + b1[e]
            h = z / (1.0 + np.exp(-z))
            yo = h @ w2[e].T + b2[e]
            out[rows] += g_e[e][C:, None] * yo

    return out.reshape(B, S, D), aux_loss


# revision 19
# speedup vs baseline: 1.0933x; 1.0186x over previous
"""MoE layer (B=2,S=2048,D=1024,H=2048,E=8,top-2) on 8 Trainium2 NeuronCores.

Strategy: expert-parallel. The gate (N x E logits, top-2, softmax) is tiny and
runs on host. Core e owns expert e's weights; the host gathers the tokens
routed to expert e, ships them (transposed, padded to capacity C) to core e,
each core runs the dense FFN y = silu(x @ w1.T + b1) @ w2.T + b2 on its
tokens, and the host scatter-adds gate-weighted results back.

Device layout (per core):
  xT   [D, C]   routed tokens, transposed  (lhs-free layout for matmul 1)
  w1T  [D, H]   w1[e].T
  w2T  [H, D]   w2[e].T
  b1   [128, H/128]  b1[e] chunked per-partition
  b2   [128, D/128]
  yT   [D, C]   output, transposed

Matmul 1: hT[m*128+p, n] = sum_d w1[h,d] x[n,d]  -> stationary w1T chunk,
moving xT chunk, PSUM accumulate over D. Silu+bias fused on ScalarE.
Matmul 2 symmetric with w2T/hT -> yT, Identity+bias epilogue.
All matmuls run as float32r (full PE rate for moving dim >= 256).
"""

import math
from contextlib import ExitStack

import numpy as np

import concourse.bass as bass
import concourse.tile as tile
from concourse import bacc, mybir
from concourse.bass_utils import run_bass_kernel_spmd

B, S, D, H, E, TOPK = 2, 2048, 1024, 2048, 8, 2
N = B * S
P = 128
TT = 512  # token tile (moving dim per matmul)
KD, KH = D // P, H // P
N_CORES = 8
BALANCE_COEFF = 0.01

_kernel_cache: dict[int, object] = {}
_TRACE = False
LAST_RESULTS = None


def _build(C: int):
    f32 = mybir.dt.float32
    f32r = mybir.dt.float32r
    act = mybir.ActivationFunctionType

    nc = bacc.Bacc("TRN2", target_bir_lowering=False, debug=False)
    xT = nc.dram_tensor("xT", [D, C], f32r, kind="ExternalInput")
    w1T = nc.dram_tensor("w1T", [D, H], f32r, kind="ExternalInput")
    w2T = nc.dram_tensor("w2T", [H, D], f32r, kind="ExternalInput")
    b1 = nc.dram_tensor("b1", [P, KH], f32, kind="ExternalInput")
    b2 = nc.dram_tensor("b2", [P, KD], f32, kind="ExternalInput")
    yT = nc.dram_tensor("yT", [D, C], f32, kind="ExternalOutput")

    xT_r = xT.rearrange("(k p) c -> k p c", p=P)
    w1T_r = w1T.rearrange("(k p) h -> k p h", p=P)
    w2T_r = w2T.rearrange("(k p) d -> k p d", p=P)
    yT_r = yT.rearrange("(k p) c -> k p c", p=P)

    # token tiles: mostly 512-wide, optional trailing 256 (C is a multiple
    # of 256; fp32r matmul needs moving dim >= 256 for full PE rate)
    assert C % 256 == 0
    tiles = [TT] * (C // TT)
    if C % TT:
        tiles.append(C % TT)
    G = 4  # PSUM bank group: h/d chunks accumulated concurrently

    with tile.TileContext(nc) as tc, ExitStack() as ctx:
        wpool = ctx.enter_context(tc.tile_pool(name="w", bufs=1))
        bpool = ctx.enter_context(tc.tile_pool(name="b", bufs=1))
        xpool = ctx.enter_context(tc.tile_pool(name="x", bufs=1))
        hpool = ctx.enter_context(tc.tile_pool(name="h", bufs=1))
        ypool = ctx.enter_context(tc.tile_pool(name="y", bufs=4))
        pspool = ctx.enter_context(tc.tile_pool(name="ps", bufs=8, space="PSUM"))

        # x tile 0 + w1 first, all as separate chunk DMAs on the sync HWDGE
        # (independent DMAs fan out across parallel hardware queues); w2
        # follows on the same path. Biases ride the gpsimd queue.
        # Issue order matters: the sync sequencer issues one DMA per ~0.7us,
        # and matmul k of group 0 is gated on xt chunk k + the first half of
        # w1 chunk k. Interleave so the PE unblocks progressively, then
        # stream the rest (w1 second halves, w2) behind it.
        xt = xpool.tile([P, KD * TT], f32r)
        tt0 = tiles[0]
        HW2, DW2 = H // 2, D // 2
        w1s = [wpool.tile([P, H], f32r, tag=f"w1_{k}", name=f"w1_{k}") for k in range(KD)]
        w2s = [wpool.tile([P, D], f32r, tag=f"w2_{k}", name=f"w2_{k}") for k in range(KH)]
        for k in range(KD):
            nc.sync.dma_start(
                xt[:, k * TT : k * TT + tt0], xT_r[k, :, 0:tt0]
            )
            nc.sync.dma_start(w1s[k][:, :HW2], w1T_r[k, :, :HW2])
        b1t = bpool.tile([P, KH], f32, tag="b1")
        nc.gpsimd.dma_start(b1t[:], b1[:])
        b2t = bpool.tile([P, KD], f32, tag="b2")
        nc.gpsimd.dma_start(b2t[:], b2[:])
        for k in range(KD):
            nc.sync.dma_start(w1s[k][:, HW2:], w1T_r[k, :, HW2:])
        for k in range(KH):
            nc.sync.dma_start(w2s[k][:, :DW2], w2T_r[k, :, :DW2])
            nc.sync.dma_start(w2s[k][:, DW2:], w2T_r[k, :, DW2:])

        # Warm-up matmuls: the PE idles ~13us waiting for the first x/w1
        # chunks; junk matmuls on a memset tile get the HAM clock gate to
        # 8/8 (2.4 GHz) before real work arrives, avoiding the ~2x-slow
        # cold phase. Results go to a never-read PSUM slot.
        warm_src = bpool.tile([P, TT], mybir.dt.bfloat16, tag="warm")
        nc.vector.memset(warm_src[:], 0.0)
        warm_ps = pspool.tile([P, TT], f32, tag="ps", name="warm_ps")
        for _ in range(20):
            nc.tensor.matmul(
                warm_ps[:], warm_src[:, :P], warm_src[:],
                start=True, stop=True,
            )

        off = 0
        for ti, tt in enumerate(tiles):
            if ti > 0:
                xt = xpool.tile([P, KD * TT], f32r)
                for k in range(KD):
                    nc.sync.dma_start(
                        xt[:, k * TT : k * TT + tt],
                        xT_r[k, :, off : off + tt],
                    )
            ht = hpool.tile([P, KH * TT], f32r)
            if ti == 0:
                # mm1 tile 0: k-outer groups -- the first matmuls only need
                # w1s[0] + xt chunk 0, so the PE starts while w1 streams in.
                for g in range(KH // G):
                    pss = [pspool.tile([P, TT], f32, tag="ps", name=f"ps_{ti}_{g}_{i}") for i in range(G)]
                    for k in range(KD):
                        for mi in range(G):
                            m = g * G + mi
                            nc.tensor.matmul(
                                pss[mi][:, :tt],
                                w1s[k][:, m * P : (m + 1) * P],
                                xt[:, k * TT : k * TT + tt],
                                start=(k == 0),
                                stop=(k == KD - 1),
                            )
                    for mi in range(G):
                        m = g * G + mi
                        nc.scalar.activation(
                            ht[:, m * TT : m * TT + tt], pss[mi][:, :tt],
                            act.Silu, bias=b1t[:, m : m + 1],
                        )
            else:
                # weights resident: chunk-at-a-time so each chunk's epilogue
                # overlaps the next chunk's matmul stream.
                for m in range(KH):
                    ps = pspool.tile([P, TT], f32, tag="ps", name=f"ps1_{ti}_{m}")
                    for k in range(KD):
                        nc.tensor.matmul(
                            ps[:, :tt],
                            w1s[k][:, m * P : (m + 1) * P],
                            xt[:, k * TT : k * TT + tt],
                            start=(k == 0),
                            stop=(k == KD - 1),
                        )
                    nc.scalar.activation(
                        ht[:, m * TT : m * TT + tt], ps[:, :tt],
                        act.Silu, bias=b1t[:, m : m + 1],
                    )
            # mm2: chunk-at-a-time for the same reason; the final chunk's
            # writeback rides the fast sync HWDGE to shorten the tail.
            last_tile = ti == len(tiles) - 1
            for j in range(KD):
                ps = pspool.tile([P, TT], f32, tag="ps", name=f"ps2_{ti}_{j}")
                for k in range(KH):
                    nc.tensor.matmul(
                        ps[:, :tt],
                        w2s[k][:, j * P : (j + 1) * P],
                        ht[:, k * TT : k * TT + tt],
                        start=(k == 0),
                        stop=(k == KH - 1),
                    )
                yt = ypool.tile([P, TT], f32)
                nc.scalar.activation(
                    yt[:, :tt], ps[:, :tt], act.Identity,
                    bias=b2t[:, j : j + 1],
                )
                eng = nc.sync if (last_tile and j >= KD - 2) else nc.gpsimd
                eng.dma_start(yT_r[j, :, off : off + tt], yt[:, :tt])
            off += tt

    nc.compile()
    return nc


def _get_kernel(C: int):
    if C not in _kernel_cache:
        _kernel_cache[C] = _build(C)
    return _kernel_cache[C]


def kernel(**inputs):
    global LAST_RESULTS
    x = np.asarray(inputs["x"], dtype=np.float32)
    gate_w = np.asarray(inputs["gate_w"], dtype=np.float32)
    w1 = np.asarray(inputs["w1"], dtype=np.float32)
    b1 = np.asarray(inputs["b1"], dtype=np.float32)
    w2 = np.asarray(inputs["w2"], dtype=np.float32)
    b2 = np.asarray(inputs["b2"], dtype=np.float32)

    xf = x.reshape(N, D)

    # --- gating on host (tiny) ---
    logits = xf.astype(np.float64) @ gate_w.T.astype(np.float64)  # [N, E]
    top_i = np.argsort(-logits, axis=1, kind="stable")[:, :TOPK]  # [N, K]
    top_v = np.take_along_axis(logits, top_i, axis=1)
    ex = np.exp(top_v - top_v[:, :1])
    gates = (ex / ex.sum(axis=1, keepdims=True)).astype(np.float32)  # [N, K]

    counts = np.bincount(top_i.ravel(), minlength=E).astype(np.float64)
    load = counts / (N * TOPK)
    aux_loss = np.float32(BALANCE_COEFF * E * np.sum(load * load))

    # --- dispatch ---
    idx_e, g_e = [], []
    for e in range(E):
        mask = top_i == e  # at most one True per row
        rows = np.nonzero(mask.any(axis=1))[0]
        idx_e.append(rows)
        g_e.append(gates[mask])  # row-major matches ascending rows

    # Capacity: smallest multiple of 256 (>=512) such that at most
    # OVERFLOW_MAX (token, expert) pairs spill; spilled pairs are computed
    # on host in fp32 (exact). Balanced loads make spill tiny or zero.
    OVERFLOW_MAX = 256
    cnts = np.array([len(r) for r in idx_e])
    C = max(512, int(math.ceil(cnts.max() / 256)) * 256)
    while C > 512 and np.maximum(cnts - (C - 256), 0).sum() <= OVERFLOW_MAX:
        C -= 256
    nc = _get_kernel(C)

    in_maps = []
    for e in range(E):
        cnt = min(len(idx_e[e]), C)
        xTe = np.zeros((D, C), dtype=np.float32)
        if cnt:
            xTe[:, :cnt] = xf[idx_e[e][:cnt]].T
        in_maps.append(
            {
                "xT": xTe,
                "w1T": np.ascontiguousarray(w1[e].T),
                "w2T": np.ascontiguousarray(w2[e].T),
                "b1": np.ascontiguousarray(b1[e].reshape(KH, P).T),
                "b2": np.ascontiguousarray(b2[e].reshape(KD, P).T),
            }
        )

    res = run_bass_kernel_spmd(
        nc, in_maps, core_ids=list(range(N_CORES)), trace=_TRACE
    )
    LAST_RESULTS = res

    # --- combine on host ---
    out = np.zeros((N, D), dtype=np.float32)
    for e in range(E):
        cnt = min(len(idx_e[e]), C)
        if cnt:
            ye = res.results[e]["yT"][:, :cnt].T  # [cnt, D]
            out[idx_e[e][:cnt]] += g_e[e][:cnt, None] * ye
        if len(idx_e[e]) > C:  # overflow pairs -> exact host FFN
            rows = idx_e[e][C:]
            z = xf[rows] @ w1[e].T + b1[e]
            h = z / (1.0 + np.exp(-z))
            yo = h @ w2[e].T + b2[e]
            out[rows] += g_e[e][C:, None] * yo

    return out.reshape(B, S, D), aux_loss
